# revision 15
# baseline (speedup 1.0000x reference)
"""ChebConv GNN kernel for Trainium2 (8 NeuronCores, data-parallel over batch).

The reference network (per graph, N=24 nodes):
  h1 = elu(sum_k Tk(L) x Wk + b1)     ChebConv K=3, 4->8
  h2 = elu(sum_k Tk(L) h1 Wk + b2)    ChebConv K=3, 8->8
  out = log_softmax(fc2(fc1(h2.flat)))

Everything is linear between the two ELUs and the final log_softmax, and the
Chebyshev propagation matrices A_k (24x24) are batch-independent.  The whole
network collapses to a per-graph MLP:
  z  = x.reshape(96)
  p1 = M1.T @ z  + b1          M1:[96,192]  = sum_k A_k (x) W1_k
  h1 = elu(p1)
  p2 = M2.T @ h1 + c2          M2:[192,192]
  h2 = elu(p2)
  d  = wd . h2   + bd2         wd:[192]  (fc2@fc1 fused, logit difference)
  out = [-softplus(d), -softplus(-d)]
M1/M2/wd are precomputed on host from edge_index + weights (all tiny).

v2 mapping (vs. the v1 feature-major pipeline at ~189us):
 * x is transposed to feature-major [97, R] bf16 ON HOST (row 96 = ones), so
   the kernel does no PE transposes and the input DMA is perfectly
   contiguous per partition.
 * Layer 1 is a standard weight-stationary matmul: lhsT = M1aug chunks
   ([97,127] and [97,65]; the ones row carries b1, the 127/65 split leaves
   partition 127 of the layer-1 output tile free for a ones row), moving =
   zt columns.  2 matmuls of 512 cols per half-tile.
 * Layer 2 swaps roles: the stationary is the ACTIVATION block [128|65, 128]
   (128 graphs), the moving is M2aug [128|65, 192].  The output lands
   BATCH-major [128 graphs, 192 features] in PSUM, so the layer-2 ELU is a
   fully lane-packed ACT op and the final dot product d = wd.h2 becomes a
   free-axis reduction on the (otherwise idle) Vector engine
   (tensor_tensor mult + tensor_reduce).  c2 rides row 127 of M2aug-a
   against the preset ones row of the activation tile.
 * All matmul operands are bf16 (full PE rate, half the DMA), PSUM stays
   f32.  ELU is a single ScalarE pass via the patched Exp activation table
   (positive-side spline buckets compute x+1), exactly as v1; the +1 shift
   is absorbed into c2 / bd2 on host.
 * Tensor work drops from ~10.2k PE cycles per 1024 graphs (transposes,
   4x512 L1, 8x512 L2, 4x512 FC) to ~5.1k (4x512 L1 + 16x192 L2, FC free).
"""

import json
import os
import shutil
import sys
import tempfile

import numpy as np

sys.path.insert(0, "/opt/trn_rl_repo")

B, N, F_IN, HID, NCLS = 131072, 24, 4, 8, 2
NCORES = 8
R = B // NCORES          # graphs per core = 16384
FIN = N * F_IN           # 96
FH = N * HID             # 192
MA = 127                 # layer-1 M-chunk a (features 0:127)
MB = FH - MA             # 65   (features 127:192)
HU = R // 512            # 32 half-tiles of 512 graph-columns
PBLK = 128               # graphs per layer-2 stationary block

_ACT_SET = "natural_log_exp_and_others"


def _prepare_act_tables() -> str:
    """Copy the stock activation tables and patch Exp's positive-side
    buckets from e^x to x+1, turning Exp into elu(x)+1.  Returns the
    path to the patched act_info.json."""
    dst = os.path.join(tempfile.gettempdir(), "bass_elu_act_tables_v1")
    marker = os.path.join(dst, ".patched_ok")
    if os.path.exists(marker):
        return os.path.join(dst, "act_info.json")

    from neuronxcc.driver.Job import Job
    from neuronxcc.driver.jobs.support.FindActInfo import findActInfoFile

    src = os.path.dirname(findActInfoFile(Job.getPackageDir(), "gen3"))
    if os.path.exists(dst):
        shutil.rmtree(dst)
    shutil.copytree(src, dst)
    for root, _, files in os.walk(dst):
        for f in files:
            os.chmod(os.path.join(root, f), 0o644)

    with open(os.path.join(dst, f"{_ACT_SET}.json")) as f:
        prof = json.load(f)
    b0 = prof["func_to_bkt_start_idx"]["exp"]
    starts = sorted(prof["func_to_bkt_start_idx"].values())
    b1 = min(s for s in starts if s > b0)

    path = os.path.join(dst, f"{_ACT_SET}_bkt.bin")
    raw = np.fromfile(path, dtype=np.float32).reshape(-1, 8).copy()
    for i in range(b0, b1):
        x0 = raw[i, 4]
        if x0 > 0.0:
            raw[i, :5] = [x0 + 1.0, 1.0, 0.0, 0.0, x0]
    raw.tofile(path)
    with open(marker, "w") as f:
        f.write("ok")
    return os.path.join(dst, "act_info.json")


def _install_ntff_hook():
    """Register the axon NTFF-profiling hook that the agent image's antenv
    package lacks, so run_bass_kernel_spmd(trace=True) can capture HW
    profiles through the tunnel."""
    if "antenv.axon_hooks" in sys.modules:
        return True
    try:
        import types

        from trn_agent_boot.trn_boot import _ntff_profile_via_ctypes

        hook = _ntff_profile_via_ctypes("/opt/axon/libaxon_pjrt.so")
        mod = types.ModuleType("antenv.axon_hooks")
        mod.get_axon_ntff_profile_hook = lambda: hook
        mod.set_axon_ntff_profile_hook = lambda h: None
        sys.modules["antenv.axon_hooks"] = mod
        return True
    except Exception as e:  # pragma: no cover - profiling is best-effort
        print("ntff hook install failed:", e)
        return False


def _patch_tile_drain():
    """This walrus build rejects TPB_CTRL instructions with more than one
    sem wait; split the TileContext tail drain into one drain per wait."""
    import concourse.tile as tile_mod
    from concourse.vector_clock import ScopedClock, VectorClock

    if getattr(tile_mod.TileContext, "_drain_patched", False):
        return

    def _drain_and_barrier(self, tick_clock, wait_clock):
        gc = tick_clock.global_clock
        n = len(gc)
        for p in range(n):
            t = gc[p]
            if t <= 0:
                continue
            vec = [0] * n
            vec[p] = t
            d = self.nc.sync.drain()
            wait_clock.add_sem_waits(d.ins, ScopedClock({None: VectorClock(vec)}))
        self.nc.all_engine_barrier()
        popped = self.nc._tile_sem_poison_stack.pop()
        assert popped is self._sem_poison
        self.nc.clear_and_free_semaphores(list(self.sems.allocated().values()))
        self.nc.all_engine_barrier()

    tile_mod.TileContext._drain_and_barrier = _drain_and_barrier
    tile_mod.TileContext._drain_patched = True


def _split_multiwaits(nc):
    """This walrus build accepts at most one sem-wait per instruction.
    Post-process the serialized BIR: for every instruction carrying N>1
    waits, insert N-1 single-wait NoOp instructions just before it on the
    same engine."""
    orig = nc.to_json_bytes

    def patched():
        m = json.loads(orig())
        counter = [0]
        for func in m["functions"]:
            for blk in func["blocks"]:
                out = []
                for inst in blk["instructions"]:
                    si = inst.get("sync_info")
                    ow = (si or {}).get("on_wait") or []
                    eng = inst.get("engine", "Unassigned")
                    if len(ow) > 1 and eng != "Unassigned":
                        for w in ow[:-1]:
                            counter[0] += 1
                            out.append({
                                "debug": inst.get("debug", 0),
                                "engine": eng,
                                "ins": [],
                                "name": f"IWS-{counter[0]}",
                                "opcode": "NoOp",
                                "outs": [],
                                "sync_info": {"on_wait": [w]},
                            })
                        si["on_wait"] = [ow[-1]]
                    out.append(inst)
                blk["instructions"] = out
        return json.dumps(m).encode()

    nc.to_json_bytes = patched


def _host_weights(edge_index, conv1_W, conv1_b, conv2_W, conv2_b,
                  fc1_W, fc1_b, fc2_W, fc2_b):
    """Fold graph propagation + all linear layers into dense matrices."""
    ei = np.asarray(edge_index)
    row, col = ei[0].astype(np.int64), ei[1].astype(np.int64)
    deg = np.zeros(N, np.float64)
    np.add.at(deg, row, 1.0)
    dis = np.where(deg > 0, deg ** -0.5, 0.0)
    ew = -dis[row] * dis[col]
    S = np.zeros((N, N), np.float64)
    np.add.at(S, (row, col), ew)

    A = np.stack([np.eye(N), S, 2.0 * (S @ S) - np.eye(N)])  # [3,24,24]

    W1 = np.asarray(conv1_W, np.float64)   # [3,4,8]
    W2 = np.asarray(conv2_W, np.float64)   # [3,8,8]
    # M1[(m,f),(n,h)] = sum_k A_k[n,m] W1_k[f,h]
    M1 = np.einsum('knm,kfh->mfnh', A, W1).reshape(FIN, FH)
    M2 = np.einsum('knm,kgh->mgnh', A, W2).reshape(FH, FH)
    b1 = np.tile(np.asarray(conv1_b, np.float64), N)          # [192]
    b2 = np.tile(np.asarray(conv2_b, np.float64), N)          # [192]

    Wf = np.asarray(fc2_W, np.float64) @ np.asarray(fc1_W, np.float64)  # [2,192]
    bf = np.asarray(fc2_W, np.float64) @ np.asarray(fc1_b, np.float64) \
        + np.asarray(fc2_b, np.float64)                                  # [2]
    wd = Wf[1] - Wf[0]
    bd = bf[1] - bf[0]

    # ELU pass returns elu(y)+1; absorb the -1 into the consumer's bias.
    c2 = b2 - M2.sum(axis=0)       # bias for layer2 given h1' = h1+1
    bd2 = bd - wd.sum()            # bias for fc given h2' = h2+1

    return (M1.astype(np.float64), b1, M2.astype(np.float64), c2,
            wd, float(bd2))


def _build_bass(bd2: float):
    import concourse.bass as bass
    import concourse.mybir as mybir
    from concourse.tile import TileContext

    _patch_tile_drain()

    f32 = mybir.dt.float32
    bf16 = mybir.dt.bfloat16
    AF = mybir.ActivationFunctionType
    ALU = mybir.AluOpType

    nc = bass.Bass(debug=False)

    zt_d = nc.dram_tensor("zt", [FIN + 1, R], bf16, kind="ExternalInput").ap()
    m1_d = nc.dram_tensor("m1", [FIN + 1, FH], bf16, kind="ExternalInput").ap()
    m2a_d = nc.dram_tensor("m2a", [MA + 1, FH], bf16, kind="ExternalInput").ap()
    m2b_d = nc.dram_tensor("m2b", [MB, FH], bf16, kind="ExternalInput").ap()
    wdr_d = nc.dram_tensor("wdr", [128, 4, FH], bf16, kind="ExternalInput").ap()
    bia_d = nc.dram_tensor("bia", [128, 2], f32, kind="ExternalInput").ap()
    out_d = nc.dram_tensor("out", [R, NCLS], f32, kind="ExternalOutput").ap()

    with TileContext(nc) as tc:
        with (
            tc.tile_pool(name="consts", bufs=1) as cpool,
            tc.tile_pool(name="h2f", bufs=3) as hpool,
            tc.tile_pool(name="tail", bufs=1) as tpool,
            tc.tile_pool(name="p1a", bufs=2, space="PSUM") as psa,
            tc.tile_pool(name="p1b", bufs=2, space="PSUM") as psb,
            tc.tile_pool(name="p2", bufs=2, space="PSUM") as ps2,
        ):
            zt = cpool.tile([FIN + 1, R], bf16)
            NDMA = 8
            CH = R // NDMA
            for g in range(NDMA):
                nc.sync.dma_start(out=zt[:, g * CH:(g + 1) * CH],
                                  in_=zt_d[:, g * CH:(g + 1) * CH])
            m1 = cpool.tile([FIN + 1, FH], bf16)
            nc.sync.dma_start(out=m1[:], in_=m1_d[:])
            m2a = cpool.tile([MA + 1, FH], bf16)
            nc.sync.dma_start(out=m2a[:], in_=m2a_d[:])
            m2b = cpool.tile([MB, FH], bf16)
            nc.sync.dma_start(out=m2b[:], in_=m2b_d[:])
            wdr = cpool.tile([128, 4, FH], bf16)
            nc.sync.dma_start(out=wdr[:], in_=wdr_d[:])
            bia = cpool.tile([128, 2], f32)   # [bd2, -bd2]
            nc.sync.dma_start(out=bia[:], in_=bia_d[:])

            dstage = tpool.tile([128, HU * 4], f32)   # fc logit-diffs + bd2

            NB = 3
            haas = [cpool.tile([128, 512], bf16, name=f"haa{i}")
                    for i in range(NB)]
            hbbs = [cpool.tile([MB, 512], bf16, name=f"hbb{i}")
                    for i in range(NB)]
            for t in haas:
                # row 127 must read 1.0 (it carries c2 into layer 2); ACT
                # overwrites rows 0:127 each round, so set the whole tile
                # once (partition-127-only memsets fail BIR verification).
                nc.gpsimd.memset(t[:], 1.0)

            live = {}

            def l1(h):
                sl = slice(512 * h, 512 * (h + 1))
                pa = psa.tile([MA, 512], f32, space="PSUM")
                pb = psb.tile([MB, 512], f32, space="PSUM")
                nc.tensor.matmul(out=pa[:], lhsT=m1[:, 0:MA], rhs=zt[:, sl],
                                 start=True, stop=True)
                nc.tensor.matmul(out=pb[:], lhsT=m1[:, MA:FH], rhs=zt[:, sl],
                                 start=True, stop=True)
                haa = haas[h % NB]
                hbb = hbbs[h % NB]
                nc.scalar.activation(haa[0:MA, :], pa[:], AF.Exp)
                nc.scalar.activation(hbb[:], pb[:], AF.Exp)
                live[h] = (haa, hbb)

            def l2(h):
                haa, hbb = live.pop(h)
                p2 = ps2.tile([128, 4, 256], f32, space="PSUM")
                for j in range(4):
                    blk = slice(PBLK * j, PBLK * (j + 1))
                    nc.tensor.matmul(out=p2[:, j, 0:FH], lhsT=haa[:, blk],
                                     rhs=m2a[:], start=True, stop=False)
                    nc.tensor.matmul(out=p2[:, j, 0:FH], lhsT=hbb[:, blk],
                                     rhs=m2b[:], start=False, stop=True)
                h2 = hpool.tile([128, 4, FH], bf16)
                fcm = hpool.tile([128, 4, FH], bf16)
                nc.scalar.activation(h2[:], p2[:, :, 0:FH], AF.Exp)
                nc.vector.tensor_tensor(out=fcm[:], in0=h2[:], in1=wdr[:],
                                        op=ALU.mult)
                nc.vector.tensor_reduce(out=dstage[:, 4 * h:4 * h + 4],
                                        in_=fcm[:], axis=mybir.AxisListType.X,
                                        op=ALU.add)

            for h in range(HU):
                l1(h)
                if h:
                    l2(h - 1)
            l2(HU - 1)

            # ---- tail: out0 = -softplus(d'), out1 = -softplus(-d') ----
            # d' = d + bd2;  softplus(y) = relu(y) + ln(1 + e^-|y|)
            t1 = tpool.tile([128, HU * 4], f32)
            t2 = tpool.tile([128, HU * 4], f32)
            t3 = tpool.tile([128, HU * 4], f32)
            ra = tpool.tile([128, HU * 4], f32)
            rb = tpool.tile([128, HU * 4], f32)
            uu = tpool.tile([128, HU * 4], f32)
            v = tpool.tile([128, HU * 4, 2], f32)
            nc.scalar.activation(t1[:], dstage[:], AF.Abs, bias=bia[:, 0:1])
            nc.scalar.activation(t2[:], t1[:], AF.Exp, scale=-1.0)
            nc.scalar.activation(t3[:], t2[:], AF.Ln, bias=1.0)
            nc.scalar.activation(ra[:], dstage[:], AF.Relu, bias=bia[:, 0:1])
            nc.scalar.activation(rb[:], dstage[:], AF.Relu, scale=-1.0,
                                 bias=bia[:, 1:2])
            nc.vector.tensor_scalar_mul(uu[:], t3[:], -1.0)
            nc.vector.tensor_tensor(out=v[:, :, 0], in0=uu[:],
                                    in1=ra[:], op=ALU.subtract)
            nc.vector.tensor_tensor(out=v[:, :, 1], in0=uu[:],
                                    in1=rb[:], op=ALU.subtract)
            dst = out_d.rearrange("(p b) c -> p b c", p=128)
            nc.sync.dma_start(out=dst, in_=v[:])

    _split_multiwaits(nc)
    return nc


def _make_inputs(**inputs):
    import ml_dtypes
    bf16 = ml_dtypes.bfloat16

    M1, b1, M2, c2, wd, bd2 = _host_weights(
        inputs["edge_index"], inputs["conv1_W"], inputs["conv1_b"],
        inputs["conv2_W"], inputs["conv2_b"], inputs["fc1_W"],
        inputs["fc1_b"], inputs["fc2_W"], inputs["fc2_b"])

    m1aug = np.vstack([M1, b1[None, :]]).astype(bf16)          # [97, 192]
    m2a = np.vstack([M2[0:MA], c2[None, :]]).astype(bf16)      # [128, 192]
    m2b = M2[MA:FH].astype(bf16)                               # [65, 192]
    wdr = np.broadcast_to(wd.astype(bf16)[None, None, :], (128, 4, FH)).copy()

    x = np.asarray(inputs["x"], np.float32).reshape(B, FIN).astype(bf16)
    bia = np.empty((128, 2), np.float32)
    bia[:, 0] = bd2
    bia[:, 1] = -bd2
    const = dict(m1=np.ascontiguousarray(m1aug),
                 m2a=np.ascontiguousarray(m2a),
                 m2b=np.ascontiguousarray(m2b), wdr=wdr, bia=bia)
    in_maps = []
    for c in range(NCORES):
        # zt col 128*b + p holds graph 128*p + b, so the d matrix that DVE
        # accumulates ends up [partition p, col b] = graph 128p + b and the
        # output DMA is contiguous per partition.
        zt = np.empty((FIN + 1, R), bf16)
        zt[0:FIN] = (x[c * R:(c + 1) * R]
                     .reshape(PBLK, PBLK, FIN).transpose(2, 1, 0)
                     .reshape(FIN, R))
        zt[FIN] = bf16(1.0)
        m = dict(const)
        m["zt"] = zt
        in_maps.append(m)
    return in_maps, float(bd2)


_LAST_RESULTS = {}


def kernel(**inputs) -> np.ndarray:
    os.environ["BASS_ACT_ROOT_JSON_PATH"] = _prepare_act_tables()
    os.environ["NEURON_FORCE_RECOMPILE"] = "1"

    from concourse.bass_utils import run_bass_kernel_spmd

    in_maps, bd2 = _make_inputs(**inputs)
    nc = _build_bass(bd2)
    trace = os.environ.get("KERNEL_TRACE", "0") == "1"
    if trace:
        trace = _install_ntff_hook()
    res = run_bass_kernel_spmd(
        nc, in_maps, core_ids=list(range(NCORES)), trace=trace,
        stitch_traces=False,
    )
    _LAST_RESULTS["exec_time_ns"] = res.exec_time_ns
    _LAST_RESULTS["mean_exec_time_ns"] = res.mean_exec_time_ns
    _LAST_RESULTS["trace"] = res.instructions_and_trace
    out = np.concatenate([r["out"] for r in res.results], axis=0)
    return out.reshape(B, 1, NCLS)


# revision 19
# speedup vs baseline: 1.2922x; 1.2922x over previous
"""ChebConv GNN kernel for Trainium2 (8 NeuronCores, data-parallel over batch).

The reference network (per graph, N=24 nodes):
  h1 = elu(sum_k Tk(L) x Wk + b1)     ChebConv K=3, 4->8
  h2 = elu(sum_k Tk(L) h1 Wk + b2)    ChebConv K=3, 8->8
  out = log_softmax(fc2(fc1(h2.flat)))

Everything is linear between the two ELUs and the final log_softmax, and the
Chebyshev propagation matrices A_k (24x24) are batch-independent.  The whole
network collapses to a per-graph MLP:
  z  = x.reshape(96)
  p1 = M1.T @ z  + b1          M1:[96,192]  = sum_k A_k (x) W1_k
  h1 = elu(p1)
  p2 = M2.T @ h1 + c2          M2:[192,192]
  h2 = elu(p2)
  d  = wd . h2   + bd2         wd:[192]  (fc2@fc1 fused, logit difference)
  out = [-softplus(d), -softplus(-d)]
M1/M2/wd are precomputed on host from edge_index + weights (all tiny).

v2 mapping (vs. the v1 feature-major pipeline at ~189us):
 * x is transposed to feature-major [97, R] bf16 ON HOST (row 96 = ones), so
   the kernel does no PE transposes and the input DMA is perfectly
   contiguous per partition.
 * Layer 1 is a standard weight-stationary matmul: lhsT = M1aug chunks
   ([97,127] and [97,65]; the ones row carries b1, the 127/65 split leaves
   partition 127 of the layer-1 output tile free for a ones row), moving =
   zt columns.  2 matmuls of 512 cols per half-tile.
 * Layer 2 swaps roles: the stationary is the ACTIVATION block [128|65, 128]
   (128 graphs), the moving is M2aug [128|65, 192].  The output lands
   BATCH-major [128 graphs, 192 features] in PSUM, so the layer-2 ELU is a
   fully lane-packed ACT op and the final dot product d = wd.h2 becomes a
   free-axis reduction on the (otherwise idle) Vector engine
   (tensor_tensor mult + tensor_reduce).  c2 rides row 127 of M2aug-a
   against the preset ones row of the activation tile.
 * All matmul operands are bf16 (full PE rate, half the DMA), PSUM stays
   f32.  ELU is a single ScalarE pass via the patched Exp activation table
   (positive-side spline buckets compute x+1), exactly as v1; the +1 shift
   is absorbed into c2 / bd2 on host.
 * Tensor work drops from ~10.2k PE cycles per 1024 graphs (transposes,
   4x512 L1, 8x512 L2, 4x512 FC) to ~5.1k (4x512 L1 + 16x192 L2, FC free).
"""

import json
import os
import shutil
import sys
import tempfile

import numpy as np

sys.path.insert(0, "/opt/trn_rl_repo")

B, N, F_IN, HID, NCLS = 131072, 24, 4, 8, 2
NCORES = 8
R = B // NCORES          # graphs per core = 16384
FIN = N * F_IN           # 96
FH = N * HID             # 192
MA = 127                 # layer-1 M-chunk a (features 0:127)
MB = FH - MA             # 65   (features 127:192)
HU = R // 512            # 32 half-tiles of 512 graph-columns
PBLK = 128               # graphs per layer-2 stationary block

_ACT_SET = "natural_log_exp_and_others"


def _prepare_act_tables() -> str:
    """Copy the stock activation tables and patch Exp's positive-side
    buckets from e^x to x+1, turning Exp into elu(x)+1.  Returns the
    path to the patched act_info.json."""
    dst = os.path.join(tempfile.gettempdir(), "bass_elu_act_tables_v1")
    marker = os.path.join(dst, ".patched_ok")
    if os.path.exists(marker):
        return os.path.join(dst, "act_info.json")

    from neuronxcc.driver.Job import Job
    from neuronxcc.driver.jobs.support.FindActInfo import findActInfoFile

    src = os.path.dirname(findActInfoFile(Job.getPackageDir(), "gen3"))
    if os.path.exists(dst):
        shutil.rmtree(dst)
    shutil.copytree(src, dst)
    for root, _, files in os.walk(dst):
        for f in files:
            os.chmod(os.path.join(root, f), 0o644)

    with open(os.path.join(dst, f"{_ACT_SET}.json")) as f:
        prof = json.load(f)
    b0 = prof["func_to_bkt_start_idx"]["exp"]
    starts = sorted(prof["func_to_bkt_start_idx"].values())
    b1 = min(s for s in starts if s > b0)

    path = os.path.join(dst, f"{_ACT_SET}_bkt.bin")
    raw = np.fromfile(path, dtype=np.float32).reshape(-1, 8).copy()
    for i in range(b0, b1):
        x0 = raw[i, 4]
        if x0 > 0.0:
            raw[i, :5] = [x0 + 1.0, 1.0, 0.0, 0.0, x0]
    raw.tofile(path)
    with open(marker, "w") as f:
        f.write("ok")
    return os.path.join(dst, "act_info.json")


def _install_ntff_hook():
    """Register the axon NTFF-profiling hook that the agent image's antenv
    package lacks, so run_bass_kernel_spmd(trace=True) can capture HW
    profiles through the tunnel."""
    if "antenv.axon_hooks" in sys.modules:
        return True
    try:
        import types

        from trn_agent_boot.trn_boot import _ntff_profile_via_ctypes

        hook = _ntff_profile_via_ctypes("/opt/axon/libaxon_pjrt.so")
        mod = types.ModuleType("antenv.axon_hooks")
        mod.get_axon_ntff_profile_hook = lambda: hook
        mod.set_axon_ntff_profile_hook = lambda h: None
        sys.modules["antenv.axon_hooks"] = mod
        return True
    except Exception as e:  # pragma: no cover - profiling is best-effort
        print("ntff hook install failed:", e)
        return False


def _patch_tile_drain():
    """This walrus build rejects TPB_CTRL instructions with more than one
    sem wait; split the TileContext tail drain into one drain per wait."""
    import concourse.tile as tile_mod
    from concourse.vector_clock import ScopedClock, VectorClock

    if getattr(tile_mod.TileContext, "_drain_patched", False):
        return

    def _drain_and_barrier(self, tick_clock, wait_clock):
        gc = tick_clock.global_clock
        n = len(gc)
        for p in range(n):
            t = gc[p]
            if t <= 0:
                continue
            vec = [0] * n
            vec[p] = t
            d = self.nc.sync.drain()
            wait_clock.add_sem_waits(d.ins, ScopedClock({None: VectorClock(vec)}))
        self.nc.all_engine_barrier()
        popped = self.nc._tile_sem_poison_stack.pop()
        assert popped is self._sem_poison
        self.nc.clear_and_free_semaphores(list(self.sems.allocated().values()))
        self.nc.all_engine_barrier()

    tile_mod.TileContext._drain_and_barrier = _drain_and_barrier
    tile_mod.TileContext._drain_patched = True


def _split_multiwaits(nc):
    """This walrus build accepts at most one sem-wait per instruction.
    Post-process the serialized BIR: for every instruction carrying N>1
    waits, insert N-1 single-wait NoOp instructions just before it on the
    same engine."""
    orig = nc.to_json_bytes

    def patched():
        m = json.loads(orig())
        counter = [0]
        for func in m["functions"]:
            for blk in func["blocks"]:
                out = []
                for inst in blk["instructions"]:
                    si = inst.get("sync_info")
                    ow = (si or {}).get("on_wait") or []
                    eng = inst.get("engine", "Unassigned")
                    if len(ow) > 1 and eng != "Unassigned":
                        for w in ow[:-1]:
                            counter[0] += 1
                            out.append({
                                "debug": inst.get("debug", 0),
                                "engine": eng,
                                "ins": [],
                                "name": f"IWS-{counter[0]}",
                                "opcode": "NoOp",
                                "outs": [],
                                "sync_info": {"on_wait": [w]},
                            })
                        si["on_wait"] = [ow[-1]]
                    out.append(inst)
                blk["instructions"] = out
        return json.dumps(m).encode()

    nc.to_json_bytes = patched


def _host_weights(edge_index, conv1_W, conv1_b, conv2_W, conv2_b,
                  fc1_W, fc1_b, fc2_W, fc2_b):
    """Fold graph propagation + all linear layers into dense matrices."""
    ei = np.asarray(edge_index)
    row, col = ei[0].astype(np.int64), ei[1].astype(np.int64)
    deg = np.zeros(N, np.float64)
    np.add.at(deg, row, 1.0)
    dis = np.where(deg > 0, deg ** -0.5, 0.0)
    ew = -dis[row] * dis[col]
    S = np.zeros((N, N), np.float64)
    np.add.at(S, (row, col), ew)

    A = np.stack([np.eye(N), S, 2.0 * (S @ S) - np.eye(N)])  # [3,24,24]

    W1 = np.asarray(conv1_W, np.float64)   # [3,4,8]
    W2 = np.asarray(conv2_W, np.float64)   # [3,8,8]
    # M1[(m,f),(n,h)] = sum_k A_k[n,m] W1_k[f,h]
    M1 = np.einsum('knm,kfh->mfnh', A, W1).reshape(FIN, FH)
    M2 = np.einsum('knm,kgh->mgnh', A, W2).reshape(FH, FH)
    b1 = np.tile(np.asarray(conv1_b, np.float64), N)          # [192]
    b2 = np.tile(np.asarray(conv2_b, np.float64), N)          # [192]

    Wf = np.asarray(fc2_W, np.float64) @ np.asarray(fc1_W, np.float64)  # [2,192]
    bf = np.asarray(fc2_W, np.float64) @ np.asarray(fc1_b, np.float64) \
        + np.asarray(fc2_b, np.float64)                                  # [2]
    wd = Wf[1] - Wf[0]
    bd = bf[1] - bf[0]

    # ELU pass returns elu(y)+1; absorb the -1 into the consumer's bias.
    c2 = b2 - M2.sum(axis=0)       # bias for layer2 given h1' = h1+1
    bd2 = bd - wd.sum()            # bias for fc given h2' = h2+1

    return (M1.astype(np.float64), b1, M2.astype(np.float64), c2,
            wd, float(bd2))


def _build_bass(bd2: float):
    import concourse.bass as bass
    import concourse.mybir as mybir
    from concourse.tile import TileContext

    _patch_tile_drain()

    f32 = mybir.dt.float32
    bf16 = mybir.dt.bfloat16
    AF = mybir.ActivationFunctionType
    ALU = mybir.AluOpType

    nc = bass.Bass(debug=False)

    zt_d = nc.dram_tensor("zt", [FIN + 1, R], bf16, kind="ExternalInput").ap()
    m1_d = nc.dram_tensor("m1", [FIN + 1, FH], bf16, kind="ExternalInput").ap()
    m2a_d = nc.dram_tensor("m2a", [MA + 1, FH], bf16, kind="ExternalInput").ap()
    m2b_d = nc.dram_tensor("m2b", [MB, FH], bf16, kind="ExternalInput").ap()
    wdr_d = nc.dram_tensor("wdr", [128, 4, FH], bf16, kind="ExternalInput").ap()
    bia_d = nc.dram_tensor("bia", [128, 2], f32, kind="ExternalInput").ap()
    out_d = nc.dram_tensor("out", [R, NCLS], f32, kind="ExternalOutput").ap()

    with TileContext(nc) as tc:
        with (
            tc.tile_pool(name="consts", bufs=1) as cpool,
            tc.tile_pool(name="h2f", bufs=3) as hpool,
            tc.tile_pool(name="tail", bufs=1) as tpool,
            tc.tile_pool(name="p1", bufs=2, space="PSUM") as psa,
            tc.tile_pool(name="p2", bufs=2, space="PSUM") as ps2,
        ):
            # weights first: the qSP DMA queue drains in order and layer-1
            # needs them before the first zt chunk completes.
            m1 = cpool.tile([FIN + 1, FH], bf16)
            nc.sync.dma_start(out=m1[:], in_=m1_d[:])
            m2a = cpool.tile([MA + 1, FH], bf16)
            nc.sync.dma_start(out=m2a[:], in_=m2a_d[:])
            m2b = cpool.tile([MB, FH], bf16)
            nc.sync.dma_start(out=m2b[:], in_=m2b_d[:])
            wdr = cpool.tile([128, 4, FH], bf16)
            nc.sync.dma_start(out=wdr[:], in_=wdr_d[:])
            bia = cpool.tile([128, 2], f32)   # [bd2, -bd2]
            nc.sync.dma_start(out=bia[:], in_=bia_d[:])
            # zt: cap descriptors at 512B so the HW DGE sprays them over all
            # 16 DMA engines (one 4KB descriptor per partition lands on a
            # single engine and runs ~20 GB/s).
            zt = cpool.tile([FIN + 1, R], bf16)
            NDMA = 8
            CH = R // NDMA
            for g in range(NDMA):
                nc.sync.dma_start(out=zt[:, g * CH:(g + 1) * CH],
                                  in_=zt_d[:, g * CH:(g + 1) * CH],
                                  max_dma_last_dim=256)

            dstage = tpool.tile([128, HU * 4], f32)   # fc logit-diffs + bd2

            NB = 3
            # hab cols 0:512 = h1 features 0:127 (+ones row 127 for c2),
            # cols 512:1024 = h1 features 127:192 in rows 0:65 (rows 65:127
            # there hold elu(garbage) and are never read).
            habs = [cpool.tile([128, 1024], bf16, name=f"hab{i}")
                    for i in range(NB)]
            for t in habs:
                # row 127 must read 1.0 (it carries c2 into layer 2); ACT
                # overwrites rows 0:127 each round, so set the whole tile
                # once (partition-127-only memsets fail BIR verification).
                nc.gpsimd.memset(t[:], 1.0)

            live = {}

            def l1(h):
                sl = slice(512 * h, 512 * (h + 1))
                pab = psa.tile([128, 1024], f32, space="PSUM")
                nc.tensor.matmul(out=pab[0:MA, 0:512], lhsT=m1[:, 0:MA],
                                 rhs=zt[:, sl], start=True, stop=True)
                nc.tensor.matmul(out=pab[0:MB, 512:1024], lhsT=m1[:, MA:FH],
                                 rhs=zt[:, sl], start=True, stop=True)
                hab = habs[h % NB]
                # one ACT covers both chunks; rows 65:127 of cols 512:1024
                # are junk but harmless.
                nc.scalar.activation(hab[0:MA, :], pab[0:MA, :], AF.Exp)
                live[h] = hab

            def l2(h):
                hab = live.pop(h)
                p2 = ps2.tile([128, 4, 256], f32, space="PSUM")
                for j in range(4):
                    blk = slice(PBLK * j, PBLK * (j + 1))
                    bblk = slice(512 + PBLK * j, 512 + PBLK * (j + 1))
                    nc.tensor.matmul(out=p2[:, j, 0:FH], lhsT=hab[:, blk],
                                     rhs=m2a[:], start=True, stop=False)
                    nc.tensor.matmul(out=p2[:, j, 0:FH], lhsT=hab[0:MB, bblk],
                                     rhs=m2b[:], start=False, stop=True)
                h2 = hpool.tile([128, 4, FH], bf16)
                fcm = hpool.tile([128, 4, FH], bf16)
                nc.scalar.activation(h2[:], p2[:, :, 0:FH], AF.Exp)
                nc.vector.tensor_tensor(out=fcm[:], in0=h2[:], in1=wdr[:],
                                        op=ALU.mult)
                nc.vector.tensor_reduce(out=dstage[:, 4 * h:4 * h + 4],
                                        in_=fcm[:], axis=mybir.AxisListType.X,
                                        op=ALU.add)

            for h in range(HU):
                l1(h)
                if h:
                    l2(h - 1)
            l2(HU - 1)

            # ---- tail: out0 = -softplus(d'), out1 = -softplus(-d') ----
            # d' = d + bd2;  softplus(y) = relu(y) + ln(1 + e^-|y|)
            t1 = tpool.tile([128, HU * 4], f32)
            t2 = tpool.tile([128, HU * 4], f32)
            t3 = tpool.tile([128, HU * 4], f32)
            ra = tpool.tile([128, HU * 4], f32)
            rb = tpool.tile([128, HU * 4], f32)
            uu = tpool.tile([128, HU * 4], f32)
            v = tpool.tile([128, HU * 4, 2], f32)
            nc.scalar.activation(t1[:], dstage[:], AF.Abs, bias=bia[:, 0:1])
            nc.scalar.activation(t2[:], t1[:], AF.Exp, scale=-1.0)
            nc.scalar.activation(t3[:], t2[:], AF.Ln, bias=1.0)
            nc.scalar.activation(ra[:], dstage[:], AF.Relu, bias=bia[:, 0:1])
            nc.scalar.activation(rb[:], dstage[:], AF.Relu, scale=-1.0,
                                 bias=bia[:, 1:2])
            nc.vector.tensor_scalar_mul(uu[:], t3[:], -1.0)
            nc.vector.tensor_tensor(out=v[:, :, 0], in0=uu[:],
                                    in1=ra[:], op=ALU.subtract)
            nc.vector.tensor_tensor(out=v[:, :, 1], in0=uu[:],
                                    in1=rb[:], op=ALU.subtract)
            dst = out_d.rearrange("(p b) c -> p b c", p=128)
            nc.sync.dma_start(out=dst, in_=v[:])

    _split_multiwaits(nc)
    return nc


def _make_inputs(**inputs):
    import ml_dtypes
    bf16 = ml_dtypes.bfloat16

    M1, b1, M2, c2, wd, bd2 = _host_weights(
        inputs["edge_index"], inputs["conv1_W"], inputs["conv1_b"],
        inputs["conv2_W"], inputs["conv2_b"], inputs["fc1_W"],
        inputs["fc1_b"], inputs["fc2_W"], inputs["fc2_b"])

    m1aug = np.vstack([M1, b1[None, :]]).astype(bf16)          # [97, 192]
    m2a = np.vstack([M2[0:MA], c2[None, :]]).astype(bf16)      # [128, 192]
    m2b = M2[MA:FH].astype(bf16)                               # [65, 192]
    wdr = np.broadcast_to(wd.astype(bf16)[None, None, :], (128, 4, FH)).copy()

    x = np.asarray(inputs["x"], np.float32).reshape(B, FIN).astype(bf16)
    bia = np.empty((128, 2), np.float32)
    bia[:, 0] = bd2
    bia[:, 1] = -bd2
    const = dict(m1=np.ascontiguousarray(m1aug),
                 m2a=np.ascontiguousarray(m2a),
                 m2b=np.ascontiguousarray(m2b), wdr=wdr, bia=bia)
    in_maps = []
    for c in range(NCORES):
        # zt col 128*b + p holds graph 128*p + b, so the d matrix that DVE
        # accumulates ends up [partition p, col b] = graph 128p + b and the
        # output DMA is contiguous per partition.
        zt = np.empty((FIN + 1, R), bf16)
        zt[0:FIN] = (x[c * R:(c + 1) * R]
                     .reshape(PBLK, PBLK, FIN).transpose(2, 1, 0)
                     .reshape(FIN, R))
        zt[FIN] = bf16(1.0)
        m = dict(const)
        m["zt"] = zt
        in_maps.append(m)
    return in_maps, float(bd2)


_LAST_RESULTS = {}


def kernel(**inputs) -> np.ndarray:
    os.environ["BASS_ACT_ROOT_JSON_PATH"] = _prepare_act_tables()
    os.environ["NEURON_FORCE_RECOMPILE"] = "1"

    from concourse.bass_utils import run_bass_kernel_spmd

    in_maps, bd2 = _make_inputs(**inputs)
    nc = _build_bass(bd2)
    trace = os.environ.get("KERNEL_TRACE", "0") == "1"
    if trace:
        trace = _install_ntff_hook()
    res = run_bass_kernel_spmd(
        nc, in_maps, core_ids=list(range(NCORES)), trace=trace,
        stitch_traces=False,
    )
    _LAST_RESULTS["exec_time_ns"] = res.exec_time_ns
    _LAST_RESULTS["mean_exec_time_ns"] = res.mean_exec_time_ns
    _LAST_RESULTS["trace"] = res.instructions_and_trace
    out = np.concatenate([r["out"] for r in res.results], axis=0)
    return out.reshape(B, 1, NCLS)
